# revision 6
# baseline (speedup 1.0000x reference)
"""DMN encoder (3-hop masked-attention message passing) on 8 trn2 cores.

Sharding: pure data-parallel over the batch dim (16 rows/core).

v4 design:
  - host pre-casts V to bf16 and pre-transposes it: device gets BOTH
    layouts (vn: neighbors-on-partitions, vt: d-on-partitions) via plain
    HWDGE DMA - no SWDGE casts, no PE transposes.
  - big DMAs issued first, split across sync+scalar HWDGE queues.
  - vs/vu dots: lhsT = vt chunk (stationary), rhs = wfu -> out [128, 2].
  - o-passes: lhsT = vn chunk (stationary), rhs = num cols -> out
    [128(d), hops] accumulated in PSUM; one PSUM->SBUF copy per group.
  - denominator epsilon term (1e-5) dropped: the masked softmax numerator
    always contains its own max, so denom >= O(0.1) and the epsilon is
    ~1e-4 relative at worst - far below tolerance. This kills the
    masked-max (m1) partition-reduce and the corr chain entirely.
  - row broadcasts ([1,G] -> [128,G]) moved off the PE queue onto the
    idle GpSimd engine (partition_broadcast).
"""
import sys

sys.path.insert(0, "/opt/trn_rl_repo")

import numpy as np
import ml_dtypes
import concourse.bass as bass
import concourse.tile as tile
from concourse import mybir
from concourse import bass_isa
from concourse.bass_utils import run_bass_kernel_spmd
from contextlib import ExitStack

N_CORES = 8
B, N, D = 128, 2048, 128
BC = B // N_CORES          # batch rows per core
CH = N // 128              # neighbor chunks of 128
GB = 8                     # batch rows per pipeline group
NG = BC // GB
AF = mybir.ActivationFunctionType
ALU = mybir.AluOpType
FP32 = mybir.dt.float32
BF16 = mybir.dt.bfloat16
CLAMP = 60.0               # overflow guard on exp() arguments

_mwctr = [0]


def _split_multiwaits(nc):
    """This walrus build rejects >1 sync-wait per instruction; hoist extras
    onto standalone EventSemaphore instructions on the same engine."""
    for fn in nc.m.functions:
        for bb in fn.blocks:
            new_list = []
            changed = False
            for ins in bb.instructions:
                si = getattr(ins, "sync_info", None)
                on_wait = list(si.on_wait) if si is not None else []
                if len(on_wait) > 1:
                    changed = True
                    for w in on_wait[:-1]:
                        _mwctr[0] += 1
                        ev = mybir.InstEventSemaphore(
                            name=f"I-mwfix-{_mwctr[0]}", ins=[], outs=[])
                        ev.engine = ins.engine
                        ev.debug = ins.debug
                        ev.sync_info = mybir.SyncInfo(on_wait=[w], on_update=[])
                        new_list.append(ev)
                        nc.register_instruction(ev, overwrite=True)
                    si.on_wait = [on_wait[-1]]
                    ins.sync_info = si
                new_list.append(ins)
            if changed:
                live = bb.instructions
                live[:] = new_list


def _build():
    nc = bass.Bass()
    vn_in = nc.dram_tensor("vn", [128, BC, CH, D], BF16, kind="ExternalInput")
    vt_in = nc.dram_tensor("vt", [128, BC, CH, 128], BF16,
                           kind="ExternalInput")
    mask_in = nc.dram_tensor("mask_t", [128, CH, BC], BF16,
                             kind="ExternalInput")
    e1_t = nc.dram_tensor("e1_t", [D, BC], FP32, kind="ExternalInput")
    w_lhsT = nc.dram_tensor("w_lhsT", [D, D], FP32, kind="ExternalInput")
    b_col = nc.dram_tensor("b_col", [D, 1], FP32, kind="ExternalInput")
    wfu_in = nc.dram_tensor("wfu", [D, 2], FP32, kind="ExternalInput")
    attb_in = nc.dram_tensor("attb", [1, 1], FP32, kind="ExternalInput")
    ident_in = nc.dram_tensor("ident", [128, 128], FP32, kind="ExternalInput")
    y = nc.dram_tensor("y", [BC, D], FP32, kind="ExternalOutput")

    with tile.TileContext(nc) as tc, ExitStack() as ctx:
        P = lambda **kw: ctx.enter_context(tc.tile_pool(**kw))
        sb = P(name="sb", bufs=1)                       # persistent singles
        wk = P(name="wk", bufs=3)                       # temporaries
        ps_vv = P(name="ps_vv", bufs=2, space="PSUM")   # vs/vu collectors
        ps_oA = P(name="ps_oA", bufs=2, space="PSUM")   # passA accumulators
        ps_oB = P(name="ps_oB", bufs=2, space="PSUM")   # passB accumulators
        ps_sm = P(name="ps_sm", bufs=2, space="PSUM")   # small matmul outs

        # ---- big V loads first (half-group slices, alternating HWDGE
        #      trigger queues) so SDMA engines saturate immediately ----
        vt_sb = sb.tile([128, BC, CH, 128], BF16, tag="vt")
        vn_sb = sb.tile([128, BC, CH, D], BF16, tag="vn")
        HG = GB // 2
        for g in range(NG):
            for h in range(2):
                hsl = slice(g * GB + h * HG, g * GB + (h + 1) * HG)
                nc.sync.dma_start(out=vt_sb[:, hsl, :, :],
                                  in_=vt_in[:, hsl, :, :])
                nc.scalar.dma_start(out=vn_sb[:, hsl, :, :],
                                    in_=vn_in[:, hsl, :, :])

        # ---- small params ----
        w_sb = sb.tile([D, D], FP32, tag="w_sb")
        nc.scalar.dma_start(out=w_sb, in_=w_lhsT[:, :])
        bcol_sb = sb.tile([D, 1], FP32, tag="bcol")
        nc.scalar.dma_start(out=bcol_sb, in_=b_col[:, :])
        wfu_sb = sb.tile([D, 2], FP32, tag="wfu")
        nc.sync.dma_start(out=wfu_sb, in_=wfu_in[:, :])
        attb_sb = sb.tile([1, 1], FP32, tag="attb")
        nc.sync.dma_start(out=attb_sb, in_=attb_in[:, :])
        identf = sb.tile([128, 128], FP32, tag="identf")
        nc.scalar.dma_start(out=identf, in_=ident_in[:, :])
        u0 = sb.tile([D, BC], FP32, tag="u0")
        nc.sync.dma_start(out=u0, in_=e1_t[:, :])
        mask_sb = sb.tile([128, CH, BC], BF16, tag="mask")
        nc.sync.dma_start(out=mask_sb, in_=mask_in[:, :, :])

        wfu_bf = sb.tile([D, 2], BF16, tag="wfub")
        nc.vector.tensor_copy(wfu_bf, wfu_sb)

        vsvu = sb.tile([128, CH, BC, 2], FP32, tag="vsvu")
        E = sb.tile([128, CH, BC], BF16, tag="E")
        num01 = sb.tile([128, CH, BC, 2], BF16, tag="num01")
        num2 = sb.tile([128, CH, BC, 1], BF16, tag="num2")
        o01 = sb.tile([128, BC, 2], FP32, tag="o01")
        o2 = sb.tile([128, BC], FP32, tag="o2")

        # ---- helpers (b-group sliced) ----
        def dot_wu(rhs_tile):
            ps = ps_sm.tile([1, GB], FP32, tag="sm")
            nc.tensor.matmul(ps, lhsT=wfu_sb[:, 1:2], rhs=rhs_tile,
                             start=True, stop=True)
            return ps

        def bcast_row(src_1xg, tg):
            ps = ps_sm.tile([128, GB], FP32, tag="sm")
            nc.tensor.matmul(ps, lhsT=ones_row, rhs=src_1xg,
                             start=True, stop=True)
            return ps

        def colsum(red_tile):
            ps = ps_sm.tile([1, GB], FP32, tag="sm")
            nc.tensor.matmul(ps, lhsT=ones_col, rhs=red_tile,
                             start=True, stop=True)
            return ps

        def hop_t(c_sb, tg):
            tcl = wk.tile([1, GB], FP32, tag=f"hs_t{tg}")
            nc.vector.tensor_scalar_min(tcl, c_sb, CLAMP)
            texp = wk.tile([1, GB], FP32, tag=f"texp{tg}")
            nc.scalar.activation(out=texp, in_=tcl, func=AF.Exp)
            return bcast_row(texp, tg)

        def make_num(t_bc, num_out_view, Eg, maskg, tg):
            tmp = wk.tile([128, CH, GB], BF16, tag=f"numt{tg}")
            nc.vector.tensor_tensor(
                out=tmp, in0=Eg,
                in1=bass.AP(tensor=t_bc.tensor, offset=t_bc.offset,
                            ap=[t_bc.ap[0], [0, CH], t_bc.ap[1]]),
                op=ALU.mult)
            nc.vector.tensor_scalar_max(tmp, tmp, 1.0)
            nc.vector.tensor_tensor(
                out=num_out_view, in0=tmp, in1=maskg, op=ALU.mult)
            return num_out_view

        def denom(num_view, tg):
            red = wk.tile([128, GB], FP32, tag=f"dred{tg}")
            nc.vector.tensor_reduce(
                out=red, in_=num_view.rearrange("p c b -> p b c"),
                axis=mybir.AxisListType.X, op=ALU.add)
            ps = colsum(red)
            recip = wk.tile([1, GB], FP32, tag=f"recip{tg}")
            nc.vector.reciprocal(recip, ps)
            return recip

        def weighted_sum_vu(num_view, vu_g, recip, tg):
            nv = wk.tile([128, CH, GB], FP32, tag=f"nv{tg}")
            nc.vector.tensor_tensor(out=nv, in0=num_view, in1=vu_g,
                                    op=ALU.mult)
            red = wk.tile([128, GB], FP32, tag=f"nvred{tg}")
            nc.vector.tensor_reduce(
                out=red, in_=nv.rearrange("p c b -> p b c"),
                axis=mybir.AxisListType.X, op=ALU.add)
            ps = colsum(red)
            out = wk.tile([1, GB], FP32, tag=f"owu{tg}")
            nc.vector.tensor_tensor(out=out, in0=ps, in1=recip, op=ALU.mult)
            return out

        def lin_relu(u_tile, tg):
            ps = ps_sm.tile([D, GB], FP32, tag="sm")
            nc.tensor.matmul(ps, lhsT=w_sb, rhs=u_tile, start=True, stop=True)
            ub = wk.tile([D, GB], FP32, tag=f"ub{tg}")
            nc.scalar.activation(out=ub, in_=ps, func=AF.Relu,
                                 bias=bcol_sb, scale=1.0)
            return ub

        def make_c(base_u, owu, tg):
            ps = dot_wu(base_u)
            c_sb = wk.tile([1, GB], FP32, tag=f"c{tg}")
            nc.vector.tensor_scalar(
                out=c_sb, in0=ps, scalar1=attb_sb, scalar2=None,
                op0=ALU.add)
            if owu is not None:
                nc.vector.tensor_tensor(out=c_sb, in0=c_sb, in1=owu,
                                        op=ALU.add)
            return c_sb

        ones_col = sb.tile([128, 1], FP32, tag="onesc")
        nc.vector.memset(ones_col, 1.0)
        ones_row = sb.tile([1, 128], FP32, tag="onesr")
        nc.vector.memset(ones_row, 1.0)

        # ---- phase functions ----
        def vsvu_phase(g0, gn):
            for b in range(g0, g0 + gn):
                acc = ps_vv.tile([128, 2 * CH], FP32, tag="accv")
                for c in range(CH):
                    nc.tensor.matmul(
                        acc[:, c * 2:(c + 1) * 2],
                        lhsT=vt_sb[:, b, c, :], rhs=wfu_bf,
                        start=True, stop=True)
                if b % 2 == 0:
                    nc.vector.tensor_copy(
                        vsvu[:, :, b, :],
                        acc.rearrange("p (c h) -> p c h", h=2))
                else:
                    nc.scalar.activation(
                        out=vsvu[:, :, b, :],
                        in_=acc.rearrange("p (c h) -> p c h", h=2),
                        func=AF.Copy)

        def chain01(g0, gn, gi):
            gsl = slice(g0, g0 + gn)
            maskg = mask_sb[:, :, gsl]
            vs_g = vsvu[:, :, gsl, 0]
            vu_g = vsvu[:, :, gsl, 1]
            Eg = E[:, :, gsl]
            nc.scalar.activation(out=Eg, in_=vs_g, func=AF.Exp)

            u0g = u0[:, gsl]
            c0 = make_c(u0g, None, f"0_{gi}")
            t0bc = hop_t(c0, f"0_{gi}")
            nb0 = make_num(t0bc, num01[:, :, gsl, 0], Eg, maskg, f"0_{gi}")
            recip0 = denom(nb0, f"0_{gi}")
            o0wu = weighted_sum_vu(nb0, vu_g, recip0, f"0_{gi}")

            ub0 = lin_relu(u0g, f"0_{gi}")
            c1 = make_c(ub0, o0wu, f"1_{gi}")
            t1bc = hop_t(c1, f"1_{gi}")
            nb1 = make_num(t1bc, num01[:, :, gsl, 1], Eg, maskg, f"1_{gi}")
            recip1 = denom(nb1, f"1_{gi}")
            o1wu = weighted_sum_vu(nb1, vu_g, recip1, f"1_{gi}")
            return dict(ub0=ub0, recip0=recip0, recip1=recip1, o1wu=o1wu,
                        maskg=maskg, Eg=Eg, gsl=gsl, gi=gi)

        def passA(g0, gn):
            acc = ps_oA.tile([128, 2 * GB], FP32, tag="acca")
            for b in range(g0, g0 + gn):
                bb = b - g0
                for c in range(CH):
                    nc.tensor.matmul(
                        acc[:, bb * 2:(bb + 1) * 2],
                        lhsT=vn_sb[:, b, c, :], rhs=num01[:, c, b, :],
                        start=(c == 0), stop=(c == CH - 1))
            nc.vector.tensor_copy(
                o01[:, g0:g0 + gn, :],
                acc.rearrange("p (b h) -> p b h", h=2))

        def chain2(st):
            gsl = st["gsl"]
            gi = st["gi"]
            u1 = wk.tile([D, GB], FP32, tag="u1")
            r0bc = bcast_row(st["recip0"], f"r0_{gi}")
            nc.vector.tensor_tensor(out=u1, in0=o01[:, gsl, 0],
                                    in1=r0bc, op=ALU.mult)
            nc.vector.tensor_tensor(out=u1, in0=u1, in1=st["ub0"],
                                    op=ALU.add)
            ub1 = lin_relu(u1, f"1_{gi}")
            c2 = make_c(ub1, st["o1wu"], f"2_{gi}")
            t2bc = hop_t(c2, f"2_{gi}")
            nb2 = make_num(t2bc, num2[:, :, gsl, 0], st["Eg"], st["maskg"],
                           f"2_{gi}")
            recip2 = denom(nb2, f"2_{gi}")

            u2 = wk.tile([D, GB], FP32, tag="u2")
            r1bc = bcast_row(st["recip1"], f"r1_{gi}")
            nc.vector.tensor_tensor(out=u2, in0=o01[:, gsl, 1],
                                    in1=r1bc, op=ALU.mult)
            nc.vector.tensor_tensor(out=u2, in0=u2, in1=ub1, op=ALU.add)
            ub2 = lin_relu(u2, f"2_{gi}")
            st.update(ub2=ub2, recip2=recip2)

        def passB(g0, gn):
            acc = ps_oB.tile([128, GB], FP32, tag="accb")
            for b in range(g0, g0 + gn):
                bb = b - g0
                for c in range(CH):
                    nc.tensor.matmul(
                        acc[:, bb:bb + 1],
                        lhsT=vn_sb[:, b, c, :], rhs=num2[:, c, b, :],
                        start=(c == 0), stop=(c == CH - 1))
            nc.scalar.activation(out=o2[:, g0:g0 + gn], in_=acc,
                                 func=AF.Copy)

        def finish(st, g0, gn):
            gsl = st["gsl"]
            u3 = wk.tile([D, GB], FP32, tag="u3")
            r2bc = bcast_row(st["recip2"], f"r2_{st['gi']}")
            nc.vector.tensor_tensor(out=u3, in0=o2[:, gsl], in1=r2bc,
                                    op=ALU.mult)
            nc.vector.tensor_tensor(out=u3, in0=u3, in1=st["ub2"],
                                    op=ALU.add)
            ps_y = ps_sm.tile([GB, 128], FP32, tag="sm")
            nc.tensor.transpose(out=ps_y, in_=u3, identity=identf)
            yg = wk.tile([GB, 128], FP32, tag="yg")
            nc.vector.tensor_copy(yg, ps_y)
            nc.sync.dma_start(out=y[g0:g0 + gn, :], in_=yg)

        # ---- grouped software pipeline ----
        sts = []
        for g in range(NG):
            vsvu_phase(g * GB, GB)
            sts.append(chain01(g * GB, GB, g))
            passA(g * GB, GB)
        for g in range(NG):
            chain2(sts[g])
            passB(g * GB, GB)
            finish(sts[g], g * GB, GB)

    _split_multiwaits(nc)
    return nc


_nc_cache = None


def _get_nc():
    global _nc_cache
    if _nc_cache is None:
        _nc_cache = _build()
    return _nc_cache


def make_in_maps(inputs):
    e1 = np.asarray(inputs["e1_embeded"], dtype=np.float32)
    value = np.asarray(inputs["nei_embeded_value"], dtype=np.float32)
    mask = np.asarray(inputs["nei_mask"], dtype=np.float32)
    linfc_w = np.asarray(inputs["linfc_w"], dtype=np.float32)
    linfc_b = np.asarray(inputs["linfc_b"], dtype=np.float32)
    attfc_w = np.asarray(inputs["attfc_w"], dtype=np.float32)
    attfc_b = np.asarray(inputs["attfc_b"], dtype=np.float32)

    bf16 = ml_dtypes.bfloat16
    w_lhsT = np.ascontiguousarray(linfc_w.T)
    b_col = np.ascontiguousarray(linfc_b.reshape(D, 1))
    wfu = np.ascontiguousarray(
        np.stack([attfc_w[0, :D], attfc_w[0, D:]], axis=1))
    attb = np.asarray(attfc_b, dtype=np.float32).reshape(1, 1)
    ident = np.eye(128, dtype=np.float32)

    in_maps = []
    for core in range(N_CORES):
        b0 = core * BC
        r = value[b0:b0 + BC].reshape(BC, 128, CH, D)
        in_maps.append({
            "vn": r.transpose(1, 0, 2, 3).astype(bf16),
            "vt": r.transpose(3, 0, 2, 1).astype(bf16),
            "mask_t": mask[b0:b0 + BC].reshape(BC, 128, CH)
                      .transpose(1, 2, 0).astype(bf16),
            "e1_t": np.ascontiguousarray(e1[b0:b0 + BC].T),
            "w_lhsT": w_lhsT,
            "b_col": b_col,
            "wfu": wfu,
            "attb": attb,
            "ident": ident,
        })
    return in_maps


def kernel(**inputs):
    in_maps = make_in_maps(inputs)
    nc = _get_nc()
    res = run_bass_kernel_spmd(nc, in_maps, list(range(N_CORES)))
    out = np.concatenate([res.results[i]["y"] for i in range(N_CORES)], axis=0)
    return out.astype(np.float32)


# revision 9
# speedup vs baseline: 1.4977x; 1.4977x over previous
"""DMN encoder (3-hop masked-attention message passing) on 8 trn2 cores.

Sharding: pure data-parallel over the batch dim (16 rows/core).

v5 design:
  - host pre-casts V to fp8e4m3 in BOTH layouts (vn: neighbors-on-
    partitions, vt: d-on-partitions): 8.4 MB HBM per core, plain HWDGE
    DMA, no SWDGE casts, no PE transposes. wfu is pre-scaled by 16 and
    quantized to fp8 (its entries ~N(0, 1/256) would hit subnormals);
    the 1/16 descale is folded into the PSUM->SBUF copy.
  - DMA order on the sync FIFO ring: tiny params first, then vt row-
    sliced (compute starts after the first row lands), then vn per
    group. fp8 weights get FWL (4x weight-load rate).
  - softmax numerators are scaled by recip*128 and cast to fp8, so the
    o-passes are fp8 x fp8 and their PSUM output descaled by 1/128 IS
    the final o - the u-updates become plain adds (no recip broadcast).
  - denominator epsilon (1e-5) dropped: the masked-softmax numerator
    always contains its own max, denom >= O(0.1); error ~1e-4 relative,
    far under tolerance. Kills the masked-max partition-reduce chain.
  - u-only small matmuls (u.wu dot, linfc of u0) hoisted before the
    vsvu phases to shorten the per-group chain critical path.
"""
import sys

sys.path.insert(0, "/opt/trn_rl_repo")

import numpy as np
import ml_dtypes
import concourse.bass as bass
import concourse.tile as tile
from concourse import mybir
from concourse.bass_utils import run_bass_kernel_spmd
from contextlib import ExitStack

N_CORES = 8
B, N, D = 128, 2048, 128
BC = B // N_CORES          # batch rows per core
CH = N // 128              # neighbor chunks of 128
GB = 8                     # batch rows per pipeline group
NG = BC // GB
AF = mybir.ActivationFunctionType
ALU = mybir.AluOpType
FP32 = mybir.dt.float32
BF16 = mybir.dt.bfloat16
FP8 = mybir.dt.float8e4
CLAMP = 60.0               # overflow guard on exp() arguments
WS = 16.0                  # wfu pre-scale before fp8 quantization
P8 = 128.0                 # softmax-numerator fp8 scale

_mwctr = [0]


def _split_multiwaits(nc):
    """This walrus build rejects >1 sync-wait per instruction; hoist extras
    onto standalone EventSemaphore instructions on the same engine."""
    for fn in nc.m.functions:
        for bb in fn.blocks:
            new_list = []
            changed = False
            for ins in bb.instructions:
                si = getattr(ins, "sync_info", None)
                on_wait = list(si.on_wait) if si is not None else []
                if len(on_wait) > 1:
                    changed = True
                    for w in on_wait[:-1]:
                        _mwctr[0] += 1
                        ev = mybir.InstEventSemaphore(
                            name=f"I-mwfix-{_mwctr[0]}", ins=[], outs=[])
                        ev.engine = ins.engine
                        ev.debug = ins.debug
                        ev.sync_info = mybir.SyncInfo(on_wait=[w], on_update=[])
                        new_list.append(ev)
                        nc.register_instruction(ev, overwrite=True)
                    si.on_wait = [on_wait[-1]]
                    ins.sync_info = si
                new_list.append(ins)
            if changed:
                live = bb.instructions
                live[:] = new_list


def _build():
    nc = bass.Bass()
    vn_in = nc.dram_tensor("vn", [128, BC, CH, D], FP8, kind="ExternalInput")
    vt_in = nc.dram_tensor("vt", [128, BC, CH, 128], FP8,
                           kind="ExternalInput")
    mask_in = nc.dram_tensor("mask_t", [128, CH, BC], BF16,
                             kind="ExternalInput")
    e1_t = nc.dram_tensor("e1_t", [D, BC], FP32, kind="ExternalInput")
    w_lhsT = nc.dram_tensor("w_lhsT", [D, D], FP32, kind="ExternalInput")
    b_col = nc.dram_tensor("b_col", [D, 1], FP32, kind="ExternalInput")
    wfu_in = nc.dram_tensor("wfu", [D, 2], FP32, kind="ExternalInput")
    wfu8_in = nc.dram_tensor("wfu8", [D, 2], FP8, kind="ExternalInput")
    attb_in = nc.dram_tensor("attb", [1, 1], FP32, kind="ExternalInput")
    ident_in = nc.dram_tensor("ident", [128, 128], FP32, kind="ExternalInput")
    y = nc.dram_tensor("y", [BC, D], FP32, kind="ExternalOutput")

    with tile.TileContext(nc) as tc, ExitStack() as ctx:
        P = lambda **kw: ctx.enter_context(tc.tile_pool(**kw))
        sb = P(name="sb", bufs=1)                       # persistent singles
        wk = P(name="wk", bufs=3)                       # temporaries
        ps_vv = P(name="ps_vv", bufs=2, space="PSUM")   # vs/vu collectors
        ps_oA = P(name="ps_oA", bufs=2, space="PSUM")   # passA accumulators
        ps_oB = P(name="ps_oB", bufs=2, space="PSUM")   # passB accumulators
        ps_sm = P(name="ps_sm", bufs=2, space="PSUM")   # small matmul outs

        # ---- tiny params first: the sync HWDGE ring is FIFO, so these
        #      must precede the bulk V streams or compute waits on them ----
        wfu_sb = sb.tile([D, 2], FP32, tag="wfu")
        nc.sync.dma_start(out=wfu_sb, in_=wfu_in[:, :])
        wfu8_sb = sb.tile([D, 2], FP8, tag="wfu8")
        nc.sync.dma_start(out=wfu8_sb, in_=wfu8_in[:, :])
        attb_sb = sb.tile([1, 1], FP32, tag="attb")
        nc.sync.dma_start(out=attb_sb, in_=attb_in[:, :])
        u0 = sb.tile([D, BC], FP32, tag="u0")
        nc.sync.dma_start(out=u0, in_=e1_t[:, :])
        mask_sb = sb.tile([128, CH, BC], BF16, tag="mask")
        nc.sync.dma_start(out=mask_sb, in_=mask_in[:, :, :])
        w_sb = sb.tile([D, D], FP32, tag="w_sb")
        nc.scalar.dma_start(out=w_sb, in_=w_lhsT[:, :])
        bcol_sb = sb.tile([D, 1], FP32, tag="bcol")
        nc.scalar.dma_start(out=bcol_sb, in_=b_col[:, :])
        identf = sb.tile([128, 128], FP32, tag="identf")
        nc.scalar.dma_start(out=identf, in_=ident_in[:, :])

        # ---- bulk V: vt row-sliced then vn group-sliced, in need order ----
        vt_sb = sb.tile([128, BC, CH, 128], FP8, tag="vt")
        vn_sb = sb.tile([128, BC, CH, D], FP8, tag="vn")
        for g in range(NG):
            for b in range(g * GB, (g + 1) * GB):
                nc.sync.dma_start(out=vt_sb[:, b, :, :], in_=vt_in[:, b, :, :])
            gsl = slice(g * GB, (g + 1) * GB)
            nc.sync.dma_start(out=vn_sb[:, gsl, :, :], in_=vn_in[:, gsl, :, :])

        vsvu = sb.tile([128, CH, BC, 2], FP32, tag="vsvu")
        E = sb.tile([128, CH, BC], BF16, tag="E")
        num01 = sb.tile([128, CH, BC, 2], FP8, tag="num01")
        num2 = sb.tile([128, CH, BC, 1], FP8, tag="num2")
        o01 = sb.tile([128, BC, 2], FP32, tag="o01")
        o2 = sb.tile([128, BC], FP32, tag="o2")
        ones_col = sb.tile([128, 1], FP32, tag="onesc")
        nc.vector.memset(ones_col, 1.0)
        ones_row = sb.tile([1, 128], FP32, tag="onesr")
        nc.vector.memset(ones_row, 1.0)

        # ---- helpers (b-group sliced) ----
        def dot_wu(rhs_tile):
            ps = ps_sm.tile([1, GB], FP32, tag="sm")
            nc.tensor.matmul(ps, lhsT=wfu_sb[:, 1:2], rhs=rhs_tile,
                             start=True, stop=True)
            return ps

        def bcast_row(src_1xg):
            ps = ps_sm.tile([128, GB], FP32, tag="sm")
            nc.tensor.matmul(ps, lhsT=ones_row, rhs=src_1xg,
                             start=True, stop=True)
            return ps

        def colsum(red_tile):
            ps = ps_sm.tile([1, GB], FP32, tag="sm")
            nc.tensor.matmul(ps, lhsT=ones_col, rhs=red_tile,
                             start=True, stop=True)
            return ps

        def hop_t(c_sb, tg):
            tcl = wk.tile([1, GB], FP32, tag=f"hs_t{tg}")
            nc.vector.tensor_scalar_min(tcl, c_sb, CLAMP)
            texp = wk.tile([1, GB], FP32, tag=f"texp{tg}")
            nc.scalar.activation(out=texp, in_=tcl, func=AF.Exp)
            return bcast_row(texp)

        def make_num(t_bc, Eg, maskg, tg):
            """bf16 masked numerator tile (pre-scale), used for denom/owu."""
            tmp = wk.tile([128, CH, GB], BF16, tag=f"numt{tg}")
            nc.vector.tensor_tensor(
                out=tmp, in0=Eg,
                in1=bass.AP(tensor=t_bc.tensor, offset=t_bc.offset,
                            ap=[t_bc.ap[0], [0, CH], t_bc.ap[1]]),
                op=ALU.mult)
            nc.vector.tensor_scalar_max(tmp, tmp, 1.0)
            numb = wk.tile([128, CH, GB], BF16, tag=f"numb{tg}")
            nc.vector.tensor_tensor(out=numb, in0=tmp, in1=maskg,
                                    op=ALU.mult)
            return numb

        def denom(numb, tg):
            red = wk.tile([128, GB], FP32, tag=f"dred{tg}")
            nc.vector.tensor_reduce(
                out=red, in_=numb.rearrange("p c b -> p b c"),
                axis=mybir.AxisListType.X, op=ALU.add)
            ps = colsum(red)
            recip = wk.tile([1, GB], FP32, tag=f"recip{tg}")
            nc.vector.reciprocal(recip, ps)
            return recip

        def scale_num8(numb, recip, num8_view, tg):
            rs = wk.tile([1, GB], FP32, tag=f"rs{tg}")
            nc.vector.tensor_scalar_mul(rs, recip, P8)
            rsb = bcast_row(rs)
            nc.vector.tensor_tensor(
                out=num8_view, in0=numb,
                in1=bass.AP(tensor=rsb.tensor, offset=rsb.offset,
                            ap=[rsb.ap[0], [0, CH], rsb.ap[1]]),
                op=ALU.mult)

        def weighted_sum_vu(numb, vu_g, recip, tg):
            nv = wk.tile([128, CH, GB], FP32, tag=f"nv{tg}")
            nc.vector.tensor_tensor(out=nv, in0=numb, in1=vu_g,
                                    op=ALU.mult)
            red = wk.tile([128, GB], FP32, tag=f"nvred{tg}")
            nc.vector.tensor_reduce(
                out=red, in_=nv.rearrange("p c b -> p b c"),
                axis=mybir.AxisListType.X, op=ALU.add)
            ps = colsum(red)
            out = wk.tile([1, GB], FP32, tag=f"owu{tg}")
            nc.vector.tensor_tensor(out=out, in0=ps, in1=recip, op=ALU.mult)
            return out

        def lin_relu(u_tile, tg):
            ps = ps_sm.tile([D, GB], FP32, tag="sm")
            nc.tensor.matmul(ps, lhsT=w_sb, rhs=u_tile, start=True, stop=True)
            ub = wk.tile([D, GB], FP32, tag=f"ub{tg}")
            nc.scalar.activation(out=ub, in_=ps, func=AF.Relu,
                                 bias=bcol_sb, scale=1.0)
            return ub

        def make_c(dot_ps, owu, tg):
            c_sb = wk.tile([1, GB], FP32, tag=f"c{tg}")
            nc.vector.tensor_scalar(
                out=c_sb, in0=dot_ps, scalar1=attb_sb, scalar2=None,
                op0=ALU.add)
            if owu is not None:
                nc.vector.tensor_tensor(out=c_sb, in0=c_sb, in1=owu,
                                        op=ALU.add)
            return c_sb

        # ---- phase functions ----
        def prechain(g0, gn, gi):
            """u0-only PE work, hoisted before the bulk phases. Everything
            produced here is parked in SBUF immediately - PSUM tiles from
            the shared small-matmul pool must not stay live across the
            vsvu phases (buffer rotation would deadlock the PE queue)."""
            gsl = slice(g0, g0 + gn)
            u0g = u0[:, gsl]
            d0 = dot_wu(u0g)
            c0 = make_c(d0, None, f"0_{gi}")
            t0bc = hop_t(c0, f"0_{gi}")
            t0sb = wk.tile([128, GB], FP32, tag=f"t0sb_{gi}")
            nc.vector.tensor_copy(t0sb, t0bc)
            ub0 = lin_relu(u0g, f"0_{gi}")
            d1 = dot_wu(ub0)
            c1pre = make_c(d1, None, f"1p_{gi}")
            return dict(t0bc=t0sb, ub0=ub0, c1pre=c1pre, gsl=gsl, gi=gi)

        def vsvu_phase(g0, gn):
            for b in range(g0, g0 + gn):
                acc = ps_vv.tile([128, 2 * CH], FP32, tag="accv")
                for c in range(CH):
                    nc.tensor.matmul(
                        acc[:, c * 2:(c + 1) * 2],
                        lhsT=vt_sb[:, b, c, :], rhs=wfu8_sb,
                        start=True, stop=True)
                if b % 2 == 0:
                    nc.vector.tensor_scalar_mul(
                        vsvu[:, :, b, :],
                        acc.rearrange("p (c h) -> p c h", h=2), 1.0 / WS)
                else:
                    nc.scalar.activation(
                        out=vsvu[:, :, b, :],
                        in_=acc.rearrange("p (c h) -> p c h", h=2),
                        func=AF.Copy, scale=1.0 / WS)

        def chain01(st):
            gsl = st["gsl"]
            gi = st["gi"]
            maskg = mask_sb[:, :, gsl]
            vs_g = vsvu[:, :, gsl, 0]
            vu_g = vsvu[:, :, gsl, 1]
            Eg = E[:, :, gsl]
            nc.scalar.activation(out=Eg, in_=vs_g, func=AF.Exp)

            nb0 = make_num(st["t0bc"], Eg, maskg, f"0_{gi}")
            recip0 = denom(nb0, f"0_{gi}")
            scale_num8(nb0, recip0, num01[:, :, gsl, 0], f"0_{gi}")
            o0wu = weighted_sum_vu(nb0, vu_g, recip0, f"0_{gi}")

            c1 = wk.tile([1, GB], FP32, tag=f"c1_{gi}")
            nc.vector.tensor_tensor(out=c1, in0=st["c1pre"], in1=o0wu,
                                    op=ALU.add)
            t1bc = hop_t(c1, f"1_{gi}")
            nb1 = make_num(t1bc, Eg, maskg, f"1_{gi}")
            recip1 = denom(nb1, f"1_{gi}")
            scale_num8(nb1, recip1, num01[:, :, gsl, 1], f"1_{gi}")
            o1wu = weighted_sum_vu(nb1, vu_g, recip1, f"1_{gi}")
            st.update(o1wu=o1wu, maskg=maskg, Eg=Eg)

        def passA(g0, gn):
            acc = ps_oA.tile([128, 2 * GB], FP32, tag="acca")
            for b in range(g0, g0 + gn):
                bb = b - g0
                for c in range(CH):
                    nc.tensor.matmul(
                        acc[:, bb * 2:(bb + 1) * 2],
                        lhsT=vn_sb[:, b, c, :], rhs=num01[:, c, b, :],
                        start=(c == 0), stop=(c == CH - 1))
            nc.vector.tensor_scalar_mul(
                o01[:, g0:g0 + gn, :],
                acc.rearrange("p (b h) -> p b h", h=2), 1.0 / P8)

        def chain2(st):
            gsl = st["gsl"]
            gi = st["gi"]
            u1 = wk.tile([D, GB], FP32, tag="u1")
            nc.vector.tensor_tensor(out=u1, in0=o01[:, gsl, 0],
                                    in1=st["ub0"], op=ALU.add)
            ub1 = lin_relu(u1, f"1_{gi}")
            d2 = dot_wu(ub1)
            c2 = make_c(d2, st["o1wu"], f"2_{gi}")
            t2bc = hop_t(c2, f"2_{gi}")
            nb2 = make_num(t2bc, st["Eg"], st["maskg"], f"2_{gi}")
            recip2 = denom(nb2, f"2_{gi}")
            scale_num8(nb2, recip2, num2[:, :, gsl, 0], f"2_{gi}")

            u2 = wk.tile([D, GB], FP32, tag="u2")
            nc.vector.tensor_tensor(out=u2, in0=o01[:, gsl, 1],
                                    in1=ub1, op=ALU.add)
            ub2 = lin_relu(u2, f"2_{gi}")
            st.update(ub2=ub2)

        def passB(g0, gn):
            acc = ps_oB.tile([128, GB], FP32, tag="accb")
            for b in range(g0, g0 + gn):
                bb = b - g0
                for c in range(CH):
                    nc.tensor.matmul(
                        acc[:, bb:bb + 1],
                        lhsT=vn_sb[:, b, c, :], rhs=num2[:, c, b, :],
                        start=(c == 0), stop=(c == CH - 1))
            nc.scalar.activation(out=o2[:, g0:g0 + gn], in_=acc,
                                 func=AF.Copy, scale=1.0 / P8)

        def finish(st, g0, gn):
            gsl = st["gsl"]
            u3 = wk.tile([D, GB], FP32, tag="u3")
            nc.vector.tensor_tensor(out=u3, in0=o2[:, gsl],
                                    in1=st["ub2"], op=ALU.add)
            ps_y = ps_sm.tile([GB, 128], FP32, tag="sm")
            nc.tensor.transpose(out=ps_y, in_=u3, identity=identf)
            yg = wk.tile([GB, 128], FP32, tag="yg")
            nc.vector.tensor_copy(yg, ps_y)
            nc.sync.dma_start(out=y[g0:g0 + gn, :], in_=yg)

        # ---- grouped software pipeline ----
        sts = [prechain(g * GB, GB, g) for g in range(NG)]
        for g in range(NG):
            vsvu_phase(g * GB, GB)
        for g in range(NG):
            chain01(sts[g])
            passA(g * GB, GB)
        for g in range(NG):
            chain2(sts[g])
            passB(g * GB, GB)
            finish(sts[g], g * GB, GB)

    _split_multiwaits(nc)
    return nc


_nc_cache = None


def _get_nc():
    global _nc_cache
    if _nc_cache is None:
        _nc_cache = _build()
    return _nc_cache


def make_in_maps(inputs):
    e1 = np.asarray(inputs["e1_embeded"], dtype=np.float32)
    value = np.asarray(inputs["nei_embeded_value"], dtype=np.float32)
    mask = np.asarray(inputs["nei_mask"], dtype=np.float32)
    linfc_w = np.asarray(inputs["linfc_w"], dtype=np.float32)
    linfc_b = np.asarray(inputs["linfc_b"], dtype=np.float32)
    attfc_w = np.asarray(inputs["attfc_w"], dtype=np.float32)
    attfc_b = np.asarray(inputs["attfc_b"], dtype=np.float32)

    bf16 = ml_dtypes.bfloat16
    f8 = ml_dtypes.float8_e4m3
    w_lhsT = np.ascontiguousarray(linfc_w.T)
    b_col = np.ascontiguousarray(linfc_b.reshape(D, 1))
    wfu = np.ascontiguousarray(
        np.stack([attfc_w[0, :D], attfc_w[0, D:]], axis=1))
    wfu8 = (wfu * WS).astype(f8)
    attb = np.asarray(attfc_b, dtype=np.float32).reshape(1, 1)
    ident = np.eye(128, dtype=np.float32)

    in_maps = []
    for core in range(N_CORES):
        b0 = core * BC
        r = value[b0:b0 + BC].reshape(BC, 128, CH, D)
        in_maps.append({
            "vn": r.transpose(1, 0, 2, 3).astype(f8),
            "vt": r.transpose(3, 0, 2, 1).astype(f8),
            "mask_t": mask[b0:b0 + BC].reshape(BC, 128, CH)
                      .transpose(1, 2, 0).astype(bf16),
            "e1_t": np.ascontiguousarray(e1[b0:b0 + BC].T),
            "w_lhsT": w_lhsT,
            "b_col": b_col,
            "wfu": wfu,
            "wfu8": wfu8,
            "attb": attb,
            "ident": ident,
        })
    return in_maps


def kernel(**inputs):
    in_maps = make_in_maps(inputs)
    nc = _get_nc()
    res = run_bass_kernel_spmd(nc, in_maps, list(range(N_CORES)))
    out = np.concatenate([res.results[i]["y"] for i in range(N_CORES)], axis=0)
    return out.astype(np.float32)


# revision 14
# speedup vs baseline: 1.7416x; 1.1628x over previous
"""DMN encoder (3-hop masked-attention message passing) on 8 trn2 cores.

Sharding: pure data-parallel over the batch dim (16 rows/core).

v6 design (on top of v5's fp8 + host-transposed layouts):
  - the two row-groups' softmax chains are ZIPPED at op granularity, so
    each PE<->DVE round trip serves both groups and one group's DVE work
    hides under the other's stall.
  - per hop, the denominator sum and the vu-weighted sum ride ONE fused
    DVE reduce + ONE PE column-sum ([1, 2G]); the fp8 rescale row and the
    next hop's exp(c) row ride ONE PE broadcast matmul ([128, 2G]).
  - DMA: vt in 4 half-group slices then vn in 2 group slices on the sync
    FIFO ring (after the tiny params), pacing vsvu/passA starts.
"""
import sys

sys.path.insert(0, "/opt/trn_rl_repo")

import numpy as np
import ml_dtypes
import concourse.bass as bass
import concourse.tile as tile
from concourse import mybir
from concourse.bass_utils import run_bass_kernel_spmd
from contextlib import ExitStack

N_CORES = 8
B, N, D = 128, 2048, 128
BC = B // N_CORES          # batch rows per core
CH = N // 128              # neighbor chunks of 128
GB = 8                     # batch rows per pipeline group
NG = BC // GB
AF = mybir.ActivationFunctionType
ALU = mybir.AluOpType
FP32 = mybir.dt.float32
BF16 = mybir.dt.bfloat16
FP8 = mybir.dt.float8e4
CLAMP = 60.0               # overflow guard on exp() arguments
WS = 16.0                  # wfu pre-scale before fp8 quantization
P8 = 128.0                 # softmax-numerator fp8 scale

_mwctr = [0]


def _split_multiwaits(nc):
    """This walrus build rejects >1 sync-wait per instruction; hoist extras
    onto standalone EventSemaphore instructions on the same engine."""
    for fn in nc.m.functions:
        for bb in fn.blocks:
            new_list = []
            changed = False
            for ins in bb.instructions:
                si = getattr(ins, "sync_info", None)
                on_wait = list(si.on_wait) if si is not None else []
                if len(on_wait) > 1:
                    changed = True
                    for w in on_wait[:-1]:
                        _mwctr[0] += 1
                        ev = mybir.InstEventSemaphore(
                            name=f"I-mwfix-{_mwctr[0]}", ins=[], outs=[])
                        ev.engine = ins.engine
                        ev.debug = ins.debug
                        ev.sync_info = mybir.SyncInfo(on_wait=[w], on_update=[])
                        new_list.append(ev)
                        nc.register_instruction(ev, overwrite=True)
                    si.on_wait = [on_wait[-1]]
                    ins.sync_info = si
                new_list.append(ins)
            if changed:
                live = bb.instructions
                live[:] = new_list


def _build():
    nc = bass.Bass()
    vn_in = nc.dram_tensor("vn", [128, BC, CH, D], FP8, kind="ExternalInput")
    vt_in = nc.dram_tensor("vt", [128, BC, CH, 128], FP8,
                           kind="ExternalInput")
    mask_in = nc.dram_tensor("mask_t", [128, CH, BC], BF16,
                             kind="ExternalInput")
    e1_t = nc.dram_tensor("e1_t", [D, BC], FP32, kind="ExternalInput")
    w_lhsT = nc.dram_tensor("w_lhsT", [D, D], FP32, kind="ExternalInput")
    b_col = nc.dram_tensor("b_col", [D, 1], FP32, kind="ExternalInput")
    wfu_in = nc.dram_tensor("wfu", [D, 2], FP32, kind="ExternalInput")
    wfu8_in = nc.dram_tensor("wfu8", [D, 2], FP8, kind="ExternalInput")
    attb_in = nc.dram_tensor("attb", [1, 1], FP32, kind="ExternalInput")
    ident_in = nc.dram_tensor("ident", [128, 128], FP32, kind="ExternalInput")
    y = nc.dram_tensor("y", [BC, D], FP32, kind="ExternalOutput")

    with tile.TileContext(nc) as tc, ExitStack() as ctx:
        P = lambda **kw: ctx.enter_context(tc.tile_pool(**kw))
        sb = P(name="sb", bufs=1)                       # persistent singles
        wk = P(name="wk", bufs=3)                       # temporaries
        ps_vv = P(name="ps_vv", bufs=2, space="PSUM")   # vs/vu collectors
        ps_oA = P(name="ps_oA", bufs=2, space="PSUM")   # passA accumulators
        ps_oB = P(name="ps_oB", bufs=2, space="PSUM")   # passB accumulators
        ps_sm = P(name="ps_sm", bufs=2, space="PSUM")   # small matmul outs

        # ---- tiny params first: the sync HWDGE ring is FIFO, so these
        #      must precede the bulk V streams or compute waits on them ----
        wfu_sb = sb.tile([D, 2], FP32, tag="wfu")
        nc.sync.dma_start(out=wfu_sb, in_=wfu_in[:, :])
        wfu8_sb = sb.tile([D, 2], FP8, tag="wfu8")
        nc.sync.dma_start(out=wfu8_sb, in_=wfu8_in[:, :])
        attb_sb = sb.tile([1, 1], FP32, tag="attb")
        nc.sync.dma_start(out=attb_sb, in_=attb_in[:, :])
        u0 = sb.tile([D, BC], FP32, tag="u0")
        nc.sync.dma_start(out=u0, in_=e1_t[:, :])
        mask_sb = sb.tile([128, CH, BC], BF16, tag="mask")
        nc.sync.dma_start(out=mask_sb, in_=mask_in[:, :, :])
        w_sb = sb.tile([D, D], FP32, tag="w_sb")
        nc.scalar.dma_start(out=w_sb, in_=w_lhsT[:, :])
        bcol_sb = sb.tile([D, 1], FP32, tag="bcol")
        nc.scalar.dma_start(out=bcol_sb, in_=b_col[:, :])
        identf = sb.tile([128, 128], FP32, tag="identf")
        nc.scalar.dma_start(out=identf, in_=ident_in[:, :])

        # ---- bulk V in need order: vt halves pace vsvu, vn groups
        #      land just before the passes need them ----
        vt_sb = sb.tile([128, BC, CH, 128], FP8, tag="vt")
        vn_sb = sb.tile([128, BC, CH, D], FP8, tag="vn")
        HG = GB // 2
        for q in range(2 * NG):
            hsl = slice(q * HG, (q + 1) * HG)
            nc.sync.dma_start(out=vt_sb[:, hsl, :, :], in_=vt_in[:, hsl, :, :])
        for g in range(NG):
            gsl = slice(g * GB, (g + 1) * GB)
            nc.sync.dma_start(out=vn_sb[:, gsl, :, :], in_=vn_in[:, gsl, :, :])

        vsvu = sb.tile([128, CH, BC, 2], FP32, tag="vsvu")
        E = sb.tile([128, CH, BC], BF16, tag="E")
        num01 = sb.tile([128, CH, BC, 2], FP8, tag="num01")
        num2 = sb.tile([128, CH, BC, 1], FP8, tag="num2")
        o01 = sb.tile([128, BC, 2], FP32, tag="o01")
        o2 = sb.tile([128, BC], FP32, tag="o2")
        ones_col = sb.tile([128, 1], FP32, tag="onesc")
        nc.vector.memset(ones_col, 1.0)
        ones_row = sb.tile([1, 128], FP32, tag="onesr")
        nc.vector.memset(ones_row, 1.0)

        # ---- helpers ----
        def bc_ap(row_ap):
            """[*, G] -> broadcast over the CH axis for [128, CH, G] ops."""
            return bass.AP(tensor=row_ap.tensor, offset=row_ap.offset,
                           ap=[row_ap.ap[0], [0, CH], row_ap.ap[1]])

        def dot_wu(rhs_tile):
            ps = ps_sm.tile([1, GB], FP32, tag="sm")
            nc.tensor.matmul(ps, lhsT=wfu_sb[:, 1:2], rhs=rhs_tile,
                             start=True, stop=True)
            return ps

        def lin_relu(u_tile, tg):
            ps = ps_sm.tile([D, GB], FP32, tag="sm")
            nc.tensor.matmul(ps, lhsT=w_sb, rhs=u_tile, start=True, stop=True)
            ub = wk.tile([D, GB], FP32, tag=f"ub{tg}")
            nc.scalar.activation(out=ub, in_=ps, func=AF.Relu,
                                 bias=bcol_sb, scale=1.0)
            return ub

        # ---- phase functions ----
        def prechain(g0, gn, gi):
            """u0-only work, hoisted ahead of the bulk phases. PSUM tiles
            from the shared pool must not stay live across phases (buffer
            rotation would deadlock the PE queue), so results are parked
            in SBUF immediately."""
            gsl = slice(g0, g0 + gn)
            u0g = u0[:, gsl]
            d0 = dot_wu(u0g)
            c0 = wk.tile([1, GB], FP32, tag=f"c0_{gi}")
            nc.vector.tensor_scalar(out=c0, in0=d0, scalar1=attb_sb,
                                    scalar2=None, op0=ALU.add)
            nc.vector.tensor_scalar_min(c0, c0, CLAMP)
            texp0 = wk.tile([1, GB], FP32, tag=f"texp0_{gi}")
            nc.scalar.activation(out=texp0, in_=c0, func=AF.Exp)
            t0ps = ps_sm.tile([128, GB], FP32, tag="sm")
            nc.tensor.matmul(t0ps, lhsT=ones_row, rhs=texp0,
                             start=True, stop=True)
            t0sb = wk.tile([128, GB], FP32, tag=f"t0sb_{gi}")
            nc.vector.tensor_copy(t0sb, t0ps)
            ub0 = lin_relu(u0g, f"0_{gi}")
            d1 = dot_wu(ub0)
            c1pre = wk.tile([1, GB], FP32, tag=f"c1p_{gi}")
            nc.vector.tensor_scalar(out=c1pre, in0=d1, scalar1=attb_sb,
                                    scalar2=None, op0=ALU.add)
            return dict(t0bc=t0sb, ub0=ub0, c1pre=c1pre, gsl=gsl, gi=gi,
                        g0=g0, gn=gn)

        def vsvu_phase(g0, gn):
            for b in range(g0, g0 + gn):
                acc = ps_vv.tile([128, 2 * CH], FP32, tag="accv")
                for c in range(CH):
                    nc.tensor.matmul(
                        acc[:, c * 2:(c + 1) * 2],
                        lhsT=vt_sb[:, b, c, :], rhs=wfu8_sb,
                        start=True, stop=True)
                if b % 2 == 0:
                    nc.vector.tensor_scalar_mul(
                        vsvu[:, :, b, :],
                        acc.rearrange("p (c h) -> p c h", h=2), 1.0 / WS)
                else:
                    nc.scalar.activation(
                        out=vsvu[:, :, b, :],
                        in_=acc.rearrange("p (c h) -> p c h", h=2),
                        func=AF.Copy, scale=1.0 / WS)

        def hop_pair(sts, h):
            """One attention hop for both groups, ops zipped. h in {0, 1}.
            Needs st['t{h}bc'] (broadcast exp(c_h)); produces fp8-scaled
            num01[..., h]; h=0 also produces c1->t1bc, h=1 parks o1wu."""
            for st in sts:
                gsl, gi = st["gsl"], st["gi"]
                if h == 0:
                    Eg = E[:, :, gsl]
                    nc.scalar.activation(out=Eg, in_=vsvu[:, :, gsl, 0],
                                         func=AF.Exp)
                    st["Eg"] = Eg
                    st["maskg"] = mask_sb[:, :, gsl]
                    st["vu_g"] = vsvu[:, :, gsl, 1]
            for st in sts:
                gi = st["gi"]
                nb2 = wk.tile([128, CH, 2, GB], BF16, tag=f"nb2_{h}_{gi}")
                tmp = wk.tile([128, CH, GB], BF16, tag=f"numt{h}_{gi}")
                nc.vector.tensor_tensor(out=tmp, in0=st["Eg"],
                                        in1=bc_ap(st[f"t{h}bc"]),
                                        op=ALU.mult)
                nc.vector.tensor_scalar_max(tmp, tmp, 1.0)
                nc.vector.tensor_tensor(out=nb2[:, :, 0, :], in0=tmp,
                                        in1=st["maskg"], op=ALU.mult)
                nc.vector.tensor_tensor(out=nb2[:, :, 1, :],
                                        in0=nb2[:, :, 0, :],
                                        in1=st["vu_g"], op=ALU.mult)
                st["nb2"] = nb2
            for st in sts:
                red2 = wk.tile([128, 2, GB], FP32, tag=f"red2_{h}_{st['gi']}")
                nc.vector.tensor_reduce(
                    out=red2,
                    in_=st["nb2"].rearrange("p c t b -> p t b c"),
                    axis=mybir.AxisListType.X, op=ALU.add)
                st["red2"] = red2
            for st in sts:
                ps = ps_sm.tile([1, 2 * GB], FP32, tag="sm")
                nc.tensor.matmul(ps, lhsT=ones_col,
                                 rhs=st["red2"].rearrange("p t b -> p (t b)"),
                                 start=True, stop=True)
                st["sums"] = ps
            for st in sts:
                gi = st["gi"]
                recip = wk.tile([1, GB], FP32, tag=f"recip{h}_{gi}")
                nc.vector.reciprocal(recip, st["sums"][:, 0:GB])
                owu = wk.tile([1, GB], FP32, tag=f"owu{h}_{gi}")
                nc.vector.tensor_tensor(out=owu, in0=st["sums"][:, GB:2 * GB],
                                        in1=recip, op=ALU.mult)
                # joint row: [recip*P8 | exp(c_next)] broadcast in one matmul
                jn = 2 * GB if h == 0 else GB
                joint = wk.tile([1, 2 * GB], FP32, tag=f"joint{h}_{gi}")
                nc.vector.tensor_scalar_mul(joint[:, 0:GB], recip, P8)
                if h == 0:
                    c1 = wk.tile([1, GB], FP32, tag=f"c1_{gi}")
                    nc.vector.tensor_tensor(out=c1, in0=st["c1pre"],
                                            in1=owu, op=ALU.add)
                    nc.vector.tensor_scalar_min(c1, c1, CLAMP)
                    nc.scalar.activation(out=joint[:, GB:2 * GB], in_=c1,
                                         func=AF.Exp)
                else:
                    st["o1wu"] = owu
                st["joint"] = joint[:, 0:jn]
            for st in sts:
                jn = st["joint"].shape[1]
                psb = ps_sm.tile([128, 2 * GB], FP32, tag="sm")
                nc.tensor.matmul(psb[:, 0:jn], lhsT=ones_row,
                                 rhs=st["joint"], start=True, stop=True)
                st["rsb"] = psb[:, 0:GB]
                if h == 0:
                    st["t1bc"] = psb[:, GB:2 * GB]
            for st in sts:
                gsl = st["gsl"]
                nc.vector.tensor_tensor(out=num01[:, :, gsl, h],
                                        in0=st["nb2"][:, :, 0, :],
                                        in1=bc_ap(st["rsb"]), op=ALU.mult)

        def passA(g0, gn):
            acc = ps_oA.tile([128, 2 * GB], FP32, tag="acca")
            for b in range(g0, g0 + gn):
                bb = b - g0
                for c in range(CH):
                    nc.tensor.matmul(
                        acc[:, bb * 2:(bb + 1) * 2],
                        lhsT=vn_sb[:, b, c, :], rhs=num01[:, c, b, :],
                        start=(c == 0), stop=(c == CH - 1))
            nc.vector.tensor_scalar_mul(
                o01[:, g0:g0 + gn, :],
                acc.rearrange("p (b h) -> p b h", h=2), 1.0 / P8)

        def chain2_pair(sts):
            for st in sts:
                gsl, gi = st["gsl"], st["gi"]
                u1 = wk.tile([D, GB], FP32, tag=f"u1_{gi}")
                nc.vector.tensor_tensor(out=u1, in0=o01[:, gsl, 0],
                                        in1=st["ub0"], op=ALU.add)
                st["u1"] = u1
            for st in sts:
                st["ub1"] = lin_relu(st["u1"], f"1_{st['gi']}")
            for st in sts:
                st["d2"] = dot_wu(st["ub1"])
            for st in sts:
                gsl, gi = st["gsl"], st["gi"]
                u2 = wk.tile([D, GB], FP32, tag=f"u2_{gi}")
                nc.vector.tensor_tensor(out=u2, in0=o01[:, gsl, 1],
                                        in1=st["ub1"], op=ALU.add)
                st["u2"] = u2
            for st in sts:
                gi = st["gi"]
                c2 = wk.tile([1, GB], FP32, tag=f"c2_{gi}")
                nc.vector.tensor_scalar(out=c2, in0=st["d2"],
                                        scalar1=attb_sb, scalar2=None,
                                        op0=ALU.add)
                nc.vector.tensor_tensor(out=c2, in0=c2, in1=st["o1wu"],
                                        op=ALU.add)
                nc.vector.tensor_scalar_min(c2, c2, CLAMP)
                st["c2"] = c2
            for st in sts:
                texp2 = wk.tile([1, GB], FP32, tag=f"texp2_{st['gi']}")
                nc.scalar.activation(out=texp2, in_=st["c2"], func=AF.Exp)
                st["texp2"] = texp2
            for st in sts:
                psb = ps_sm.tile([128, 2 * GB], FP32, tag="sm")
                nc.tensor.matmul(psb[:, 0:GB], lhsT=ones_row,
                                 rhs=st["texp2"], start=True, stop=True)
                st["t2bc"] = psb[:, 0:GB]
            for st in sts:
                gi = st["gi"]
                nb2 = wk.tile([128, CH, 2, GB], BF16, tag=f"nb2_2_{gi}")
                tmp = wk.tile([128, CH, GB], BF16, tag=f"numt2_{gi}")
                nc.vector.tensor_tensor(out=tmp, in0=st["Eg"],
                                        in1=bc_ap(st["t2bc"]), op=ALU.mult)
                nc.vector.tensor_scalar_max(tmp, tmp, 1.0)
                nc.vector.tensor_tensor(out=nb2[:, :, 0, :], in0=tmp,
                                        in1=st["maskg"], op=ALU.mult)
                st["nb2c2"] = nb2
            for st in sts:
                red = wk.tile([128, GB], FP32, tag=f"red_2_{st['gi']}")
                nc.vector.tensor_reduce(
                    out=red,
                    in_=st["nb2c2"][:, :, 0, :].rearrange("p c b -> p b c"),
                    axis=mybir.AxisListType.X, op=ALU.add)
                st["redc2"] = red
            for st in sts:
                ps = ps_sm.tile([1, 2 * GB], FP32, tag="sm")
                nc.tensor.matmul(ps[:, 0:GB], lhsT=ones_col, rhs=st["redc2"],
                                 start=True, stop=True)
                st["sum2"] = ps
            for st in sts:
                gi = st["gi"]
                rs = wk.tile([1, GB], FP32, tag=f"rs2_{gi}")
                nc.vector.reciprocal(rs, st["sum2"][:, 0:GB])
                nc.vector.tensor_scalar_mul(rs, rs, P8)
                st["rs2"] = rs
            for st in sts:
                psb = ps_sm.tile([128, 2 * GB], FP32, tag="sm")
                nc.tensor.matmul(psb[:, 0:GB], lhsT=ones_row, rhs=st["rs2"],
                                 start=True, stop=True)
                st["rsb2"] = psb[:, 0:GB]
            for st in sts:
                gsl = st["gsl"]
                nc.vector.tensor_tensor(out=num2[:, :, gsl, 0],
                                        in0=st["nb2c2"][:, :, 0, :],
                                        in1=bc_ap(st["rsb2"]), op=ALU.mult)
            for st in sts:
                st["ub2"] = lin_relu(st["u2"], f"2_{st['gi']}")

        def passB(g0, gn):
            acc = ps_oB.tile([128, GB], FP32, tag="accb")
            for b in range(g0, g0 + gn):
                bb = b - g0
                for c in range(CH):
                    nc.tensor.matmul(
                        acc[:, bb:bb + 1],
                        lhsT=vn_sb[:, b, c, :], rhs=num2[:, c, b, :],
                        start=(c == 0), stop=(c == CH - 1))
            nc.scalar.activation(out=o2[:, g0:g0 + gn], in_=acc,
                                 func=AF.Copy, scale=1.0 / P8)

        def finish(st, g0, gn):
            gsl = st["gsl"]
            u3 = wk.tile([D, GB], FP32, tag=f"u3_{st['gi']}")
            nc.vector.tensor_tensor(out=u3, in0=o2[:, gsl],
                                    in1=st["ub2"], op=ALU.add)
            ps_y = ps_sm.tile([GB, 128], FP32, tag="sm")
            nc.tensor.transpose(out=ps_y, in_=u3, identity=identf)
            yg = wk.tile([GB, 128], FP32, tag=f"yg_{st['gi']}")
            nc.vector.tensor_copy(yg, ps_y)
            nc.sync.dma_start(out=y[g0:g0 + gn, :], in_=yg)

        # ---- pipeline ----
        sts = [prechain(g * GB, GB, g) for g in range(NG)]
        for g in range(NG):
            vsvu_phase(g * GB, GB)
        hop_pair(sts, 0)
        hop_pair(sts, 1)
        for g in range(NG):
            passA(g * GB, GB)
        chain2_pair(sts)
        for g in range(NG):
            passB(g * GB, GB)
        for g in range(NG):
            finish(sts[g], g * GB, GB)

    _split_multiwaits(nc)
    return nc


_nc_cache = None


def _get_nc():
    global _nc_cache
    if _nc_cache is None:
        _nc_cache = _build()
    return _nc_cache


def make_in_maps(inputs):
    e1 = np.asarray(inputs["e1_embeded"], dtype=np.float32)
    value = np.asarray(inputs["nei_embeded_value"], dtype=np.float32)
    mask = np.asarray(inputs["nei_mask"], dtype=np.float32)
    linfc_w = np.asarray(inputs["linfc_w"], dtype=np.float32)
    linfc_b = np.asarray(inputs["linfc_b"], dtype=np.float32)
    attfc_w = np.asarray(inputs["attfc_w"], dtype=np.float32)
    attfc_b = np.asarray(inputs["attfc_b"], dtype=np.float32)

    bf16 = ml_dtypes.bfloat16
    f8 = ml_dtypes.float8_e4m3
    w_lhsT = np.ascontiguousarray(linfc_w.T)
    b_col = np.ascontiguousarray(linfc_b.reshape(D, 1))
    wfu = np.ascontiguousarray(
        np.stack([attfc_w[0, :D], attfc_w[0, D:]], axis=1))
    wfu8 = (wfu * WS).astype(f8)
    attb = np.asarray(attfc_b, dtype=np.float32).reshape(1, 1)
    ident = np.eye(128, dtype=np.float32)

    in_maps = []
    for core in range(N_CORES):
        b0 = core * BC
        r = value[b0:b0 + BC].reshape(BC, 128, CH, D)
        in_maps.append({
            "vn": r.transpose(1, 0, 2, 3).astype(f8),
            "vt": r.transpose(3, 0, 2, 1).astype(f8),
            "mask_t": mask[b0:b0 + BC].reshape(BC, 128, CH)
                      .transpose(1, 2, 0).astype(bf16),
            "e1_t": np.ascontiguousarray(e1[b0:b0 + BC].T),
            "w_lhsT": w_lhsT,
            "b_col": b_col,
            "wfu": wfu,
            "wfu8": wfu8,
            "attb": attb,
            "ident": ident,
        })
    return in_maps


def kernel(**inputs):
    in_maps = make_in_maps(inputs)
    nc = _get_nc()
    res = run_bass_kernel_spmd(nc, in_maps, list(range(N_CORES)))
    out = np.concatenate([res.results[i]["y"] for i in range(N_CORES)], axis=0)
    return out.astype(np.float32)


# revision 25
# speedup vs baseline: 1.7937x; 1.0299x over previous
"""DMN encoder (3-hop masked-attention message passing) on 8 trn2 cores.

Sharding: pure data-parallel over the batch dim (16 rows/core).

v6 design (on top of v5's fp8 + host-transposed layouts):
  - the two row-groups' softmax chains are ZIPPED at op granularity, so
    each PE<->DVE round trip serves both groups and one group's DVE work
    hides under the other's stall.
  - per hop, the denominator sum and the vu-weighted sum ride ONE fused
    DVE reduce + ONE PE column-sum ([1, 2G]); the fp8 rescale row and the
    next hop's exp(c) row ride ONE PE broadcast matmul ([128, 2G]).
  - DMA: vt in 4 half-group slices then vn in 2 group slices on the sync
    FIFO ring (after the tiny params), pacing vsvu/passA starts.
"""
import sys

sys.path.insert(0, "/opt/trn_rl_repo")

import numpy as np
import ml_dtypes
import concourse.bass as bass
import concourse.tile as tile
from concourse import mybir
from concourse.bass_utils import run_bass_kernel_spmd
from contextlib import ExitStack

N_CORES = 8
B, N, D = 128, 2048, 128
BC = B // N_CORES          # batch rows per core
CH = N // 128              # neighbor chunks of 128
GB = 8                     # batch rows per pipeline group
NG = BC // GB
AF = mybir.ActivationFunctionType
ALU = mybir.AluOpType
FP32 = mybir.dt.float32
BF16 = mybir.dt.bfloat16
FP8 = mybir.dt.float8e4
CLAMP = 60.0               # overflow guard on exp() arguments
WS = 16.0                  # wfu pre-scale before fp8 quantization
P8 = 128.0                 # softmax-numerator fp8 scale

_mwctr = [0]


def _split_multiwaits(nc):
    """This walrus build rejects >1 sync-wait per instruction; hoist extras
    onto standalone EventSemaphore instructions on the same engine."""
    for fn in nc.m.functions:
        for bb in fn.blocks:
            new_list = []
            changed = False
            for ins in bb.instructions:
                si = getattr(ins, "sync_info", None)
                on_wait = list(si.on_wait) if si is not None else []
                if len(on_wait) > 1:
                    changed = True
                    for w in on_wait[:-1]:
                        _mwctr[0] += 1
                        ev = mybir.InstEventSemaphore(
                            name=f"I-mwfix-{_mwctr[0]}", ins=[], outs=[])
                        ev.engine = ins.engine
                        ev.debug = ins.debug
                        ev.sync_info = mybir.SyncInfo(on_wait=[w], on_update=[])
                        new_list.append(ev)
                        nc.register_instruction(ev, overwrite=True)
                    si.on_wait = [on_wait[-1]]
                    ins.sync_info = si
                new_list.append(ins)
            if changed:
                live = bb.instructions
                live[:] = new_list


def _build():
    nc = bass.Bass()
    vn_in = nc.dram_tensor("vn", [128, BC, CH, D], FP8, kind="ExternalInput")
    vt_in = nc.dram_tensor("vt", [128, BC, CH, 128], FP8,
                           kind="ExternalInput")
    mask_in = nc.dram_tensor("mask_t", [128, CH, BC], BF16,
                             kind="ExternalInput")
    e1_t = nc.dram_tensor("e1_t", [D, BC], FP32, kind="ExternalInput")
    w_lhsT = nc.dram_tensor("w_lhsT", [D, D], FP32, kind="ExternalInput")
    b_col = nc.dram_tensor("b_col", [D, 1], FP32, kind="ExternalInput")
    wfu_in = nc.dram_tensor("wfu", [D, 2], FP32, kind="ExternalInput")
    wfu8_in = nc.dram_tensor("wfu8", [D, 2], FP8, kind="ExternalInput")
    attb_in = nc.dram_tensor("attb", [1, 1], FP32, kind="ExternalInput")
    y = nc.dram_tensor("y", [D, BC], FP32, kind="ExternalOutput")

    with tile.TileContext(nc) as tc, ExitStack() as ctx:
        P = lambda **kw: ctx.enter_context(tc.tile_pool(**kw))
        sb = P(name="sb", bufs=1)                       # persistent singles
        wk = P(name="wk", bufs=3)                       # temporaries
        ps_vv = P(name="ps_vv", bufs=2, space="PSUM")   # vs/vu collectors
        ps_oA = P(name="ps_oA", bufs=2, space="PSUM")   # passA accumulators
        ps_oB = P(name="ps_oB", bufs=2, space="PSUM")   # passB accumulators
        ps_sm = P(name="ps_sm", bufs=2, space="PSUM")   # small matmul outs

        # ---- tiny params first: the sync HWDGE ring is FIFO, so these
        #      must precede the bulk V streams or compute waits on them ----
        wfu_sb = sb.tile([D, 2], FP32, tag="wfu")
        nc.sync.dma_start(out=wfu_sb, in_=wfu_in[:, :])
        wfu8_sb = sb.tile([D, 2], FP8, tag="wfu8")
        nc.sync.dma_start(out=wfu8_sb, in_=wfu8_in[:, :])
        attb_sb = sb.tile([1, 1], FP32, tag="attb")
        nc.sync.dma_start(out=attb_sb, in_=attb_in[:, :])
        u0 = sb.tile([D, BC], FP32, tag="u0")
        nc.sync.dma_start(out=u0, in_=e1_t[:, :])
        mask_sb = sb.tile([128, CH, BC], BF16, tag="mask")
        nc.sync.dma_start(out=mask_sb, in_=mask_in[:, :, :])
        w_sb = sb.tile([D, D], FP32, tag="w_sb")
        nc.scalar.dma_start(out=w_sb, in_=w_lhsT[:, :])
        bcol_sb = sb.tile([D, 1], FP32, tag="bcol")
        nc.scalar.dma_start(out=bcol_sb, in_=b_col[:, :])

        # ---- bulk V in need order: vt halves pace vsvu, vn groups
        #      land just before the passes need them ----
        vt_sb = sb.tile([128, BC, CH, 128], FP8, tag="vt")
        vn_sb = sb.tile([128, BC, CH, D], FP8, tag="vn")
        HG = GB // 2
        for q in range(2 * NG):
            hsl = slice(q * HG, (q + 1) * HG)
            nc.sync.dma_start(out=vt_sb[:, hsl, :, :], in_=vt_in[:, hsl, :, :])
        for g in range(NG):
            gsl = slice(g * GB, (g + 1) * GB)
            nc.sync.dma_start(out=vn_sb[:, gsl, :, :], in_=vn_in[:, gsl, :, :])

        vsvu = sb.tile([128, CH, BC, 2], FP32, tag="vsvu")
        E = sb.tile([128, CH, BC], BF16, tag="E")
        num01 = sb.tile([128, CH, BC, 2], FP8, tag="num01")
        num2 = sb.tile([128, CH, BC, 1], FP8, tag="num2")
        o01 = sb.tile([128, BC, 2], FP32, tag="o01")
        o2 = sb.tile([128, BC], FP32, tag="o2")
        ones_col = sb.tile([128, 1], BF16, tag="onesc")
        nc.vector.memset(ones_col, 1.0)
        ones_row = sb.tile([1, 128], FP32, tag="onesr")
        nc.vector.memset(ones_row, 1.0)

        # ---- helpers ----
        def bc_ap(row_ap):
            """[*, G] -> broadcast over the CH axis for [128, CH, G] ops."""
            return bass.AP(tensor=row_ap.tensor, offset=row_ap.offset,
                           ap=[row_ap.ap[0], [0, CH], row_ap.ap[1]])

        def dot_wu(rhs_tile):
            ps = ps_sm.tile([1, GB], FP32, tag="sm")
            nc.tensor.matmul(ps, lhsT=wfu_sb[:, 1:2], rhs=rhs_tile,
                             start=True, stop=True)
            return ps

        def lin_relu(u_tile, tg):
            ps = ps_sm.tile([D, GB], FP32, tag="sm")
            nc.tensor.matmul(ps, lhsT=w_sb, rhs=u_tile, start=True, stop=True)
            ub = wk.tile([D, GB], FP32, tag=f"ub{tg}")
            nc.scalar.activation(out=ub, in_=ps, func=AF.Relu,
                                 bias=bcol_sb, scale=1.0)
            return ub

        # ---- phase functions ----
        def prechain(g0, gn, gi):
            """u0-only work, hoisted ahead of the bulk phases. PSUM tiles
            from the shared pool must not stay live across phases (buffer
            rotation would deadlock the PE queue), so results are parked
            in SBUF immediately."""
            gsl = slice(g0, g0 + gn)
            u0g = u0[:, gsl]
            d0 = dot_wu(u0g)
            c0 = wk.tile([1, GB], FP32, tag=f"c0_{gi}")
            nc.vector.tensor_scalar(out=c0, in0=d0, scalar1=attb_sb,
                                    scalar2=None, op0=ALU.add)
            nc.vector.tensor_scalar_min(c0, c0, CLAMP)
            texp0 = wk.tile([1, GB], FP32, tag=f"texp0_{gi}")
            nc.scalar.activation(out=texp0, in_=c0, func=AF.Exp)
            t0ps = ps_sm.tile([128, GB], FP32, tag="sm")
            nc.tensor.matmul(t0ps, lhsT=ones_row, rhs=texp0,
                             start=True, stop=True)
            t0sb = wk.tile([128, GB], FP32, tag=f"t0sb_{gi}")
            nc.vector.tensor_copy(t0sb, t0ps)
            ub0 = lin_relu(u0g, f"0_{gi}")
            d1 = dot_wu(ub0)
            c1pre = wk.tile([1, GB], FP32, tag=f"c1p_{gi}")
            nc.vector.tensor_scalar(out=c1pre, in0=d1, scalar1=attb_sb,
                                    scalar2=None, op0=ALU.add)
            return dict(t0bc=t0sb, ub0=ub0, c1pre=c1pre, gsl=gsl, gi=gi,
                        g0=g0, gn=gn)

        def vsvu_phase(g0, gn, act_only):
            """act_only=True keeps all PSUM->SBUF copies off the DVE queue
            so the earlier group's softmax chain can start on DVE."""
            for b in range(g0, g0 + gn):
                acc = ps_vv.tile([128, 2 * CH], FP32, tag="accv")
                for c in range(CH):
                    nc.tensor.matmul(
                        acc[:, c * 2:(c + 1) * 2],
                        lhsT=vt_sb[:, b, c, :], rhs=wfu8_sb,
                        start=True, stop=True)
                if not act_only and b % 2 == 0:
                    nc.vector.tensor_scalar_mul(
                        vsvu[:, :, b, :],
                        acc.rearrange("p (c h) -> p c h", h=2), 1.0 / WS)
                else:
                    nc.scalar.activation(
                        out=vsvu[:, :, b, :],
                        in_=acc.rearrange("p (c h) -> p c h", h=2),
                        func=AF.Copy, scale=1.0 / WS)

        def hop_pair(sts, h):
            """One attention hop for both groups, ops zipped. h in {0, 1}.
            Needs st['t{h}bc'] (broadcast exp(c_h)); produces fp8-scaled
            num01[..., h]; h=0 also produces c1->t1bc, h=1 parks o1wu."""
            for st in sts:
                gsl, gi = st["gsl"], st["gi"]
                if h == 0:
                    # E holds exp(vs) * mask; the relu floor then becomes
                    # num = max(E*t, mask) - one fewer DVE op per hop
                    Eg = E[:, :, gsl]
                    nc.scalar.activation(out=Eg, in_=vsvu[:, :, gsl, 0],
                                         func=AF.Exp)
                    st["Eg"] = Eg
                    st["maskg"] = mask_sb[:, :, gsl]
                    nc.vector.tensor_tensor(out=Eg, in0=Eg, in1=st["maskg"],
                                            op=ALU.mult)
                    st["vu_g"] = vsvu[:, :, gsl, 1]
            for st in sts:
                gi = st["gi"]
                nb2 = wk.tile([128, CH, 2, GB], BF16, tag=f"nb2_{h}_{gi}")
                tmp = wk.tile([128, CH, GB], BF16, tag=f"numt{h}_{gi}")
                nc.vector.tensor_tensor(out=tmp, in0=st["Eg"],
                                        in1=bc_ap(st[f"t{h}bc"]),
                                        op=ALU.mult)
                nc.vector.tensor_tensor(out=nb2[:, :, 0, :], in0=tmp,
                                        in1=st["maskg"], op=ALU.max)
                nc.vector.tensor_tensor(out=nb2[:, :, 1, :],
                                        in0=nb2[:, :, 0, :],
                                        in1=st["vu_g"], op=ALU.mult)
                st["nb2"] = nb2
            for st in sts:
                ps = ps_sm.tile([1, 2 * GB], FP32, tag="sm")
                for c in range(CH):
                    nc.tensor.matmul(
                        ps, lhsT=ones_col,
                        rhs=st["nb2"][:, c, :, :].rearrange("p t b -> p (t b)"),
                        start=(c == 0), stop=(c == CH - 1))
                st["sums"] = ps
            for st in sts:
                gi = st["gi"]
                recip = wk.tile([1, GB], FP32, tag=f"recip{h}_{gi}")
                nc.vector.reciprocal(recip, st["sums"][:, 0:GB])
                owu = wk.tile([1, GB], FP32, tag=f"owu{h}_{gi}")
                nc.vector.tensor_tensor(out=owu, in0=st["sums"][:, GB:2 * GB],
                                        in1=recip, op=ALU.mult)
                # joint row: [recip*P8 | exp(c_next)] broadcast in one matmul
                jn = 2 * GB if h == 0 else GB
                joint = wk.tile([1, 2 * GB], FP32, tag=f"joint{h}_{gi}")
                nc.vector.tensor_scalar_mul(joint[:, 0:GB], recip, P8)
                if h == 0:
                    c1 = wk.tile([1, GB], FP32, tag=f"c1_{gi}")
                    nc.vector.tensor_tensor(out=c1, in0=st["c1pre"],
                                            in1=owu, op=ALU.add)
                    nc.vector.tensor_scalar_min(c1, c1, CLAMP)
                    nc.scalar.activation(out=joint[:, GB:2 * GB], in_=c1,
                                         func=AF.Exp)
                else:
                    st["o1wu"] = owu
                st["joint"] = joint[:, 0:jn]
            for st in sts:
                jn = st["joint"].shape[1]
                psb = ps_sm.tile([128, 2 * GB], FP32, tag="sm")
                nc.tensor.matmul(psb[:, 0:jn], lhsT=ones_row,
                                 rhs=st["joint"], start=True, stop=True)
                st["rsb"] = psb[:, 0:GB]
                if h == 0:
                    st["t1bc"] = psb[:, GB:2 * GB]
            for st in sts:
                gsl = st["gsl"]
                nc.vector.tensor_tensor(out=num01[:, :, gsl, h],
                                        in0=st["nb2"][:, :, 0, :],
                                        in1=bc_ap(st["rsb"]), op=ALU.mult)

        def passA(g0, gn):
            acc = ps_oA.tile([128, 2 * GB], FP32, tag="acca")
            for b in range(g0, g0 + gn):
                bb = b - g0
                for c in range(CH):
                    nc.tensor.matmul(
                        acc[:, bb * 2:(bb + 1) * 2],
                        lhsT=vn_sb[:, b, c, :], rhs=num01[:, c, b, :],
                        start=(c == 0), stop=(c == CH - 1))
            nc.vector.tensor_scalar_mul(
                o01[:, g0:g0 + gn, :],
                acc.rearrange("p (b h) -> p b h", h=2), 1.0 / P8)

        def chain2_pair(sts):
            for st in sts:
                gsl, gi = st["gsl"], st["gi"]
                u1 = wk.tile([D, GB], FP32, tag=f"u1_{gi}")
                nc.vector.tensor_tensor(out=u1, in0=o01[:, gsl, 0],
                                        in1=st["ub0"], op=ALU.add)
                st["u1"] = u1
            for st in sts:
                st["ub1"] = lin_relu(st["u1"], f"1_{st['gi']}")
            for st in sts:
                st["d2"] = dot_wu(st["ub1"])
            for st in sts:
                gsl, gi = st["gsl"], st["gi"]
                u2 = wk.tile([D, GB], FP32, tag=f"u2_{gi}")
                nc.vector.tensor_tensor(out=u2, in0=o01[:, gsl, 1],
                                        in1=st["ub1"], op=ALU.add)
                st["u2"] = u2
            for st in sts:
                gi = st["gi"]
                c2 = wk.tile([1, GB], FP32, tag=f"c2_{gi}")
                nc.vector.tensor_scalar(out=c2, in0=st["d2"],
                                        scalar1=attb_sb, scalar2=None,
                                        op0=ALU.add)
                nc.vector.tensor_tensor(out=c2, in0=c2, in1=st["o1wu"],
                                        op=ALU.add)
                nc.vector.tensor_scalar_min(c2, c2, CLAMP)
                st["c2"] = c2
            for st in sts:
                texp2 = wk.tile([1, GB], FP32, tag=f"texp2_{st['gi']}")
                nc.scalar.activation(out=texp2, in_=st["c2"], func=AF.Exp)
                st["texp2"] = texp2
            for st in sts:
                psb = ps_sm.tile([128, 2 * GB], FP32, tag="sm")
                nc.tensor.matmul(psb[:, 0:GB], lhsT=ones_row,
                                 rhs=st["texp2"], start=True, stop=True)
                st["t2bc"] = psb[:, 0:GB]
            for st in sts:
                gi = st["gi"]
                nb2 = wk.tile([128, CH, GB], BF16, tag=f"nb2_2_{gi}")
                nc.vector.tensor_tensor(out=nb2, in0=st["Eg"],
                                        in1=bc_ap(st["t2bc"]), op=ALU.mult)
                nc.vector.tensor_tensor(out=nb2, in0=nb2,
                                        in1=st["maskg"], op=ALU.max)
                st["nb2c2"] = nb2
            for st in sts:
                ps = ps_sm.tile([1, 2 * GB], FP32, tag="sm")
                for c in range(CH):
                    nc.tensor.matmul(ps[:, 0:GB], lhsT=ones_col,
                                     rhs=st["nb2c2"][:, c, :],
                                     start=(c == 0), stop=(c == CH - 1))
                st["sum2"] = ps
            for st in sts:
                gi = st["gi"]
                rs = wk.tile([1, GB], FP32, tag=f"rs2_{gi}")
                nc.vector.reciprocal(rs, st["sum2"][:, 0:GB])
                nc.vector.tensor_scalar_mul(rs, rs, P8)
                st["rs2"] = rs
            for st in sts:
                psb = ps_sm.tile([128, 2 * GB], FP32, tag="sm")
                nc.tensor.matmul(psb[:, 0:GB], lhsT=ones_row, rhs=st["rs2"],
                                 start=True, stop=True)
                st["rsb2"] = psb[:, 0:GB]
            for st in sts:
                gsl = st["gsl"]
                nc.vector.tensor_tensor(out=num2[:, :, gsl, 0],
                                        in0=st["nb2c2"],
                                        in1=bc_ap(st["rsb2"]), op=ALU.mult)
            for st in sts:
                st["ub2"] = lin_relu(st["u2"], f"2_{st['gi']}")

        def passB(g0, gn):
            acc = ps_oB.tile([128, GB], FP32, tag="accb")
            for b in range(g0, g0 + gn):
                bb = b - g0
                for c in range(CH):
                    nc.tensor.matmul(
                        acc[:, bb:bb + 1],
                        lhsT=vn_sb[:, b, c, :], rhs=num2[:, c, b, :],
                        start=(c == 0), stop=(c == CH - 1))
            nc.scalar.activation(out=o2[:, g0:g0 + gn], in_=acc,
                                 func=AF.Copy, scale=1.0 / P8)

        def finish(st, g0, gn):
            gsl = st["gsl"]
            u3 = wk.tile([D, GB], FP32, tag=f"u3_{st['gi']}")
            nc.vector.tensor_tensor(out=u3, in0=o2[:, gsl],
                                    in1=st["ub2"], op=ALU.add)
            nc.sync.dma_start(out=y[:, gsl], in_=u3)

        # ---- pipeline ----
        sts = [prechain(g * GB, GB, g) for g in range(NG)]
        for g in range(NG):
            vsvu_phase(g * GB, GB, act_only=(g > 0))
        hop_pair(sts, 0)
        hop_pair(sts, 1)
        for g in range(NG):
            passA(g * GB, GB)
        chain2_pair(sts)
        for g in range(NG):
            passB(g * GB, GB)
        for g in range(NG):
            finish(sts[g], g * GB, GB)

    _split_multiwaits(nc)
    return nc


_nc_cache = None


def _get_nc():
    global _nc_cache
    if _nc_cache is None:
        _nc_cache = _build()
    return _nc_cache


def make_in_maps(inputs):
    e1 = np.asarray(inputs["e1_embeded"], dtype=np.float32)
    value = np.asarray(inputs["nei_embeded_value"], dtype=np.float32)
    mask = np.asarray(inputs["nei_mask"], dtype=np.float32)
    linfc_w = np.asarray(inputs["linfc_w"], dtype=np.float32)
    linfc_b = np.asarray(inputs["linfc_b"], dtype=np.float32)
    attfc_w = np.asarray(inputs["attfc_w"], dtype=np.float32)
    attfc_b = np.asarray(inputs["attfc_b"], dtype=np.float32)

    bf16 = ml_dtypes.bfloat16
    f8 = ml_dtypes.float8_e4m3
    w_lhsT = np.ascontiguousarray(linfc_w.T)
    b_col = np.ascontiguousarray(linfc_b.reshape(D, 1))
    wfu = np.ascontiguousarray(
        np.stack([attfc_w[0, :D], attfc_w[0, D:]], axis=1))
    wfu8 = (wfu * WS).astype(f8)
    attb = np.asarray(attfc_b, dtype=np.float32).reshape(1, 1)

    in_maps = []
    for core in range(N_CORES):
        b0 = core * BC
        r = value[b0:b0 + BC].reshape(BC, 128, CH, D)
        in_maps.append({
            "vn": r.transpose(1, 0, 2, 3).astype(f8),
            "vt": r.transpose(3, 0, 2, 1).astype(f8),
            "mask_t": mask[b0:b0 + BC].reshape(BC, 128, CH)
                      .transpose(1, 2, 0).astype(bf16),
            "e1_t": np.ascontiguousarray(e1[b0:b0 + BC].T),
            "w_lhsT": w_lhsT,
            "b_col": b_col,
            "wfu": wfu,
            "wfu8": wfu8,
            "attb": attb,
        })
    return in_maps


def kernel(**inputs):
    in_maps = make_in_maps(inputs)
    nc = _get_nc()
    res = run_bass_kernel_spmd(nc, in_maps, list(range(N_CORES)))
    out = np.concatenate(
        [np.asarray(res.results[i]["y"]).T for i in range(N_CORES)], axis=0)
    return np.ascontiguousarray(out, dtype=np.float32)
